# revision 1
# baseline (speedup 1.0000x reference)
"""Self-contained Trainium2 Bass kernel: GPT-2-style causal attention block.

reference:  qkv = X @ Wqkv + b; causal softmax attention (16 heads, hd=64);
            out = A @ Wproj + bproj.   Shapes: X [4, 2048, 1024].

Sharding over 8 NeuronCores: core c -> batch-group bg=c//4 (2 batches each),
head-group hg=c%4 (4 heads each). Each core computes its heads' attention and
a partial projection; the host sums the 4 head-group partials per batch and
adds the projection bias.

Device layout highlights:
  - X is host-transposed per batch (xt [d, seq]) so QKV matmuls contract over
    d on partitions directly; q,k are produced transposed [cols, seq], v in
    natural [seq, cols] layout.
  - scores^T [sk, sq] per head via K=64 row-tiled matmul pairs (2 heads share
    the 128x128 PE array).
  - softmax without max-subtraction (scores are O(1) here); exp on ScalarE
    straight out of PSUM with the 1/sqrt(hd) scale folded in; causal masking
    by 0/1 mask multiply on the 4 diagonal-block patterns only.
  - AV with lhsT = [1 | 0*63 | v] so each head's denominator lands in PSUM
    row 0 (readable by the custom-DVE reciprocal) and o^T in rows 64-127.
  - fp32r (TF32-class) matmuls throughout: ~4x faster than fp32, ~1.5e-4 rel.
"""

import os
import sys

sys.path.insert(0, "/opt/trn_rl_repo")

import numpy as np
from contextlib import ExitStack

import concourse.bass as bass  # noqa: F401
from concourse import bacc, bass_utils
import concourse.mybir as mybir
import concourse.tile as tile

F32 = mybir.dt.float32
F32R = mybir.dt.float32r

B, S, D, H, HD = 4, 2048, 1024, 16, 64
SQC = 512   # sq chunk (psum bank free size)
SKC = 128   # sk chunk (partition dim)
N_CORES = 8
B_LOC = 2   # batches per core
NH = 4      # heads per core
NP = NH // 2
QKCOLS = NH * HD            # 256
NSC = S // SQC              # 4
NSS = S // SKC              # 16
NDC = D // 128              # 8
NCC = 2 * QKCOLS // 128     # 4


def _build(mask_engine="split"):
    nc = bacc.Bacc("TRN2", target_bir_lowering=False, debug=False)

    xt_d = nc.dram_tensor("xt", [B_LOC, D, S], F32R, kind="ExternalInput").ap()
    w1qk_d = nc.dram_tensor("w1qk", [D, 2 * QKCOLS], F32R, kind="ExternalInput").ap()
    w1v_d = nc.dram_tensor("w1v", [D, QKCOLS], F32R, kind="ExternalInput").ap()
    b1qk_d = nc.dram_tensor("b1qk", [128, NCC], F32, kind="ExternalInput").ap()
    b1v_d = nc.dram_tensor("b1v", [128, QKCOLS], F32, kind="ExternalInput").ap()
    w2_d = nc.dram_tensor("w2", [QKCOLS, D], F32R, kind="ExternalInput").ap()
    mask_d = nc.dram_tensor("masks", [4, 128, SQC], F32, kind="ExternalInput").ap()
    part_d = nc.dram_tensor("part", [B_LOC, S, D], F32, kind="ExternalOutput").ap()

    with tile.TileContext(nc) as tc, ExitStack() as ctx:
        const = ctx.enter_context(tc.tile_pool(name="const", bufs=1))
        xtp = ctx.enter_context(tc.tile_pool(name="xtp", bufs=2))
        qkp = ctx.enter_context(tc.tile_pool(name="qkp", bufs=1))
        pp = ctx.enter_context(tc.tile_pool(name="pp", bufs=4))
        np_ = ctx.enter_context(tc.tile_pool(name="npool", bufs=2))
        atp = ctx.enter_context(tc.tile_pool(name="atp", bufs=2))
        outp = ctx.enter_context(tc.tile_pool(name="outp", bufs=3))
        ps_s = ctx.enter_context(tc.tile_pool(name="ps_s", bufs=2, space="PSUM"))
        ps_o = ctx.enter_context(tc.tile_pool(name="ps_o", bufs=2, space="PSUM"))
        ps_p = ctx.enter_context(tc.tile_pool(name="ps_p", bufs=1, space="PSUM"))

        w1qk = const.tile([128, NDC, 2 * QKCOLS], F32R)
        nc.sync.dma_start(w1qk[:], w1qk_d.rearrange("(o p) c -> p o c", p=128))
        w1v = const.tile([128, NDC, QKCOLS], F32R)
        nc.sync.dma_start(w1v[:], w1v_d.rearrange("(o p) c -> p o c", p=128))
        w2sb = const.tile([128, NP, D], F32R)
        nc.sync.dma_start(w2sb[:], w2_d.rearrange("(o p) d -> p o d", p=128))
        b1qk = const.tile([128, NCC], F32)
        nc.sync.dma_start(b1qk[:], b1qk_d)
        b1v = const.tile([128, QKCOLS], F32)
        nc.sync.dma_start(b1v[:], b1v_d)
        masks = const.tile([128, 4, SQC], F32)
        nc.sync.dma_start(masks[:], mask_d.rearrange("r p f -> p r f"))

        # vaug: [sk 128, NSS, NH, 128]: col0 = 1 (denominator), 1-63 = 0,
        # 64-127 = v  (so AV psum rows: 0 = sum(p), 64-127 = o^T)
        vaug = const.tile([128, NSS, NH, 128], F32R)
        nc.vector.memset(vaug[:, :, :, 0:64].bitcast(F32), 0.0)
        nc.vector.memset(vaug[:, :, :, 0:1].bitcast(F32), 1.0)

        for b in range(B_LOC):
            # ---------------- QKV ----------------
            qkT = qkp.tile([128, 2, NP, S], F32R)  # [q/k, hp, seq]
            for sc in range(NSC):
                xt_t = xtp.tile([128, NDC, SQC], F32R)
                nc.sync.dma_start(
                    xt_t[:],
                    xt_d[b, :, sc * SQC:(sc + 1) * SQC].rearrange(
                        "(o p) s -> p o s", p=128),
                )
                for cc in range(NCC):
                    qk_ps = ps_o.tile([128, SQC], F32, tag="qk")
                    for dc in range(NDC):
                        nc.tensor.matmul(
                            qk_ps[:],
                            w1qk[:, dc, cc * 128:(cc + 1) * 128],
                            xt_t[:, dc, :],
                            start=(dc == 0), stop=(dc == NDC - 1),
                        )
                    t, hp = divmod(cc, NP)
                    nc.vector.tensor_scalar_add(
                        qkT[:, t, hp, sc * SQC:(sc + 1) * SQC],
                        qk_ps[:], b1qk[:, cc:cc + 1],
                    )
                for ss in range(4):
                    skc = sc * 4 + ss
                    v_ps = ps_p.tile([128, 2, SQC], F32, tag="pj")
                    vv = v_ps[:, 0, 0:QKCOLS]
                    for dc in range(NDC):
                        nc.tensor.matmul(
                            vv,
                            xt_t[:, dc, ss * 128:(ss + 1) * 128],
                            w1v[:, dc, :],
                            start=(dc == 0), stop=(dc == NDC - 1),
                        )
                    nc.vector.tensor_add(
                        vaug[:, skc, :, 64:128],
                        vv.rearrange("p (h x) -> p h x", x=64),
                        b1v.rearrange("p (h x) -> p h x", x=64),
                    )

            # ------------- attention + projection -------------
            for sqc in range(NSC):
                n_skc = 4 * sqc + 4
                at = atp.tile([128, NP, SQC], F32R)
                for hp in range(NP):
                    ov = [ps_o.tile([128, SQC], F32, tag="qk", name=f"ov{i}")
                          for i in range(2)]
                    for skp in range(n_skc // 2):
                        for h01 in range(2):
                            r0 = h01 * 64
                            s2 = ps_s.tile([128, 2, SQC], F32, tag="sc")
                            p2 = pp.tile([128, 2, SQC], F32R, tag="p2")
                            for j in range(2):
                                skc = skp * 2 + j
                                nc.tensor.matmul(
                                    s2[:, j, :],
                                    qkT[r0:r0 + 64, 1, hp,
                                        skc * 128:(skc + 1) * 128],
                                    qkT[r0:r0 + 64, 0, hp,
                                        sqc * SQC:(sqc + 1) * SQC],
                                    start=True, stop=True,
                                    tile_position=(r0, 0),
                                )
                            nc.scalar.activation(
                                p2[:, :, :], s2[:, :, :],
                                mybir.ActivationFunctionType.Exp, scale=0.125)
                            h = 2 * hp + h01
                            for j in range(2):
                                skc = skp * 2 + j
                                r = skc - 4 * sqc
                                if r >= 0:
                                    eng = nc.vector if (
                                        mask_engine == "dve"
                                        or (mask_engine == "split" and r < 2)
                                    ) else nc.gpsimd
                                    eng.tensor_mul(
                                        p2[:, j, :], p2[:, j, :],
                                        masks[:, r, :])
                                nc.tensor.matmul(
                                    ov[h01][:],
                                    vaug[:, skc, h, :],
                                    p2[:, j, :],
                                    start=(skc == 0),
                                    stop=(skc == n_skc - 1),
                                )
                    for h01 in range(2):
                        rsb = np_.tile([1, SQC], F32, tag="rsb")
                        nc.vector.reciprocal_approx_fast(
                            rsb[0:1, :], ov[h01][0:1, :])
                        rbc = np_.tile([128, SQC], F32, tag="rbc")
                        nc.gpsimd.partition_broadcast(rbc[:], rsb[0:1, :])
                        if h01 == 1:
                            nc.vector.tensor_mul(
                                at[64:128, hp, :], ov[h01][64:128, :],
                                rbc[64:128, :])
                        else:
                            att = np_.tile([128, SQC], F32R, tag="att")
                            nc.vector.tensor_mul(
                                att[64:128, :], ov[h01][64:128, :],
                                rbc[64:128, :])
                            nc.sync.dma_start(at[0:64, hp, :], att[64:128, :])
                for sub in range(4):
                    pj = ps_p.tile([128, 2, SQC], F32, tag="pj")
                    for half in range(2):
                        for hp in range(NP):
                            nc.tensor.matmul(
                                pj[:, half, :],
                                at[:, hp, sub * 128:(sub + 1) * 128],
                                w2sb[:, hp, half * 512:(half + 1) * 512],
                                start=(hp == 0), stop=(hp == NP - 1),
                            )
                    osb = outp.tile([128, 2, SQC], F32)
                    nc.vector.tensor_copy(osb[:], pj[:])
                    nc.sync.dma_start(
                        part_d[b, sqc * SQC + sub * 128:
                               sqc * SQC + (sub + 1) * 128, :],
                        osb[:].rearrange("p a b -> p (a b)"),
                    )

    nc.compile()
    return nc


def _make_masks():
    r = np.arange(4)[:, None, None]
    p = np.arange(128)[None, :, None]
    f = np.arange(SQC)[None, None, :]
    return (f >= 128 * r + p).astype(np.float32)


def _shard_inputs(hidden_states, c_attn_w, c_attn_b):
    masks = _make_masks()
    in_maps = []
    for c in range(N_CORES):
        bg, hg = divmod(c, 4)
        q = slice(256 * hg, 256 * hg + 256)
        k = slice(D + 256 * hg, D + 256 * hg + 256)
        v = slice(2 * D + 256 * hg, 2 * D + 256 * hg + 256)
        in_maps.append({
            "xt": np.ascontiguousarray(
                hidden_states[2 * bg:2 * bg + 2].transpose(0, 2, 1)),
            "w1qk": np.ascontiguousarray(
                np.concatenate([c_attn_w[:, q], c_attn_w[:, k]], axis=1)),
            "w1v": np.ascontiguousarray(c_attn_w[:, v]),
            "b1qk": np.ascontiguousarray(
                np.concatenate([c_attn_b[q], c_attn_b[k]]).reshape(NCC, 128).T),
            "b1v": np.tile(np.asarray(c_attn_b[v])[None, :], (128, 1)),
            "w2": None,  # filled below to keep key order stable
            "masks": masks,
        })
    return in_maps


_NC_CACHE = {}
LAST_RESULTS = None


def kernel(hidden_states, c_attn_w, c_attn_b, c_proj_w, c_proj_b):
    global LAST_RESULTS
    hidden_states = np.asarray(hidden_states, dtype=np.float32)
    c_attn_w = np.asarray(c_attn_w, dtype=np.float32)
    c_attn_b = np.asarray(c_attn_b, dtype=np.float32)
    c_proj_w = np.asarray(c_proj_w, dtype=np.float32)
    c_proj_b = np.asarray(c_proj_b, dtype=np.float32)

    if "nc" not in _NC_CACHE:
        _NC_CACHE["nc"] = _build()
    nc = _NC_CACHE["nc"]

    in_maps = _shard_inputs(hidden_states, c_attn_w, c_attn_b)
    for c in range(N_CORES):
        hg = c % 4
        in_maps[c]["w2"] = np.ascontiguousarray(
            c_proj_w[256 * hg:256 * hg + 256, :])

    trace = os.environ.get("BASS_KERNEL_TRACE", "") == "1"
    res = bass_utils.run_bass_kernel_spmd(
        nc, in_maps, core_ids=list(range(N_CORES)), trace=trace)
    LAST_RESULTS = res

    out = np.zeros((B, S, D), dtype=np.float64)
    for c in range(N_CORES):
        bg = c // 4
        out[2 * bg:2 * bg + 2] += res.results[c]["part"].astype(np.float64)
    out += c_proj_b.astype(np.float64)
    return out.astype(np.float32)
